# revision 25
# baseline (speedup 1.0000x reference)
"""MoE layer (8 experts, top-2) on 8 Trainium2 NeuronCores.

Strategy (v2): hidden-dim-split expert parallelism in bf16.
  - Host computes gating + top-2 routing (mirrors the reference ops).
  - Experts are sorted by token load and paired hot-with-cold; pair p is
    assigned to cores (2p, 2p+1), each core computing one HALF of the FFN
    hidden dim (2048 of 4096) for BOTH experts of the pair. This balances
    the per-core matmul work to ~(L_hot+L_cold)/2 token-columns regardless
    of routing skew, while keeping per-core weight traffic identical to
    one full expert (weights stream from HBM exactly once).
  - All matmul operands are bf16 (PE rate is identical to fp32r, but DMA
    bytes and SBUF footprint halve); PSUM accumulation is fp32 and the
    partial outputs return as fp32.
  - Host sums the two half partials, applies combine weights, and
    scatter-adds into token order.

Device layout: activations are transposed ([feature, token]); x lives in
SBUF as [128, 8, cap] (k-subtile middle), h as [128, 16, cap] bf16 which
fits residently, so phase 2 needs no hidden-dim quartering and y needs no
multi-pass accumulation: each phase-2 psum group covers the full 2048
contraction and evacuates straight to the output DMA.
"""

import numpy as np

N_EMBED = 1024
FFN_HIDDEN = 4096
NUM_EXPERTS = 8
TOP_K = 2
P = 128
KT1 = N_EMBED // P          # 8  k-tiles, phase 1
FH = FFN_HIDDEN // 2        # 2048 hidden features per core (half)
MT1 = FH // P               # 16 m-tiles, phase 1 (half hidden)
KT2 = FH // P               # 16 k-tiles, phase 2
MT2 = N_EMBED // P          # 8  m-tiles, phase 2

LAST_RESULT = None          # BassKernelResults of the most recent run


def _ensure_axon_hooks():
    """Make `antenv.axon_hooks` importable so BASS_TRACE=1 degrades
    gracefully instead of crashing when the image lacks the module."""
    try:
        import antenv.axon_hooks  # noqa: F401
        return
    except ImportError:
        pass
    import sys
    import types

    m = types.ModuleType("antenv.axon_hooks")
    m._hook = None
    m.set_axon_ntff_profile_hook = lambda h: setattr(m, "_hook", h)
    m.get_axon_ntff_profile_hook = lambda: m._hook
    sys.modules["antenv.axon_hooks"] = m
    try:
        from trn_agent_boot.trn_boot import _ntff_profile_via_ctypes

        m.set_axon_ntff_profile_hook(_ntff_profile_via_ctypes("/opt/axon/libaxon_pjrt.so"))
    except Exception:
        pass


def _route(x2d, Wg, bg):
    """Top-2 gating. Mirrors the reference (jax softmax + lax.top_k) so the
    selected experts match it exactly; numpy fallback is numerically
    equivalent up to fp32 rounding."""
    try:
        import jax
        import jax.numpy as jnp

        gate = jax.nn.softmax(jnp.asarray(x2d) @ jnp.asarray(Wg) + jnp.asarray(bg), axis=-1)
        scores, idx = jax.lax.top_k(gate, TOP_K)
        scores = np.asarray(scores, dtype=np.float32)
        idx = np.asarray(idx)
    except Exception:
        logits = x2d @ Wg + bg
        m = logits.max(-1, keepdims=True)
        e = np.exp(logits - m)
        p = e / e.sum(-1, keepdims=True)
        order = np.argsort(-p, axis=-1, kind="stable")
        idx = order[:, :TOP_K]
        scores = np.take_along_axis(p, idx, axis=-1)
    scores = scores / (scores.sum(-1, keepdims=True) + 1e-8)
    return idx.astype(np.int64), scores.astype(np.float32)


def _chunks(width, reverse=False):
    """Split a token capacity into matmul free-dim chunks of at most 512
    (one PSUM bank of fp32). Smallest chunk first so the kernel's very
    first psum group is short; reverse=True puts it last (for the tail)."""
    out, rem = [], width
    while rem > 0:
        c = min(rem, 512)
        out.append(c)
        rem -= c
    out.sort(reverse=reverse)
    res, off = [], 0
    for c in out:
        res.append((off, c))
        off += c
    return res


def _build_device_program(capA, capB):
    import concourse.tile as tile
    from concourse import bacc, mybir
    from concourse.tile_rust import add_dep_helper

    f32 = mybir.dt.float32
    bf16 = mybir.dt.bfloat16
    gelu = mybir.ActivationFunctionType.Gelu_apprx_tanh

    nc = bacc.Bacc("TRN2", target_bir_lowering=False, debug=False,
                   num_devices=NUM_EXPERTS)

    dram = {}
    for tag, cap in (("a", capA), ("b", capB)):
        # x is stored chunk-major: contiguous [P, KT1*cw] blocks per chunk,
        # so every chunk DMA moves contiguous per-partition runs at full
        # bandwidth (a strided whole-tensor DMA measured ~60% slower and
        # stalled the first phase for ~12us).
        dram[f"xg{tag}"] = nc.dram_tensor(f"xg{tag}", [P, KT1 * cap], bf16,
                                          kind="ExternalInput").ap()
        dram[f"w1{tag}"] = nc.dram_tensor(f"w1{tag}", [MT1, P, KT1 * P], bf16,
                                          kind="ExternalInput").ap()
        dram[f"w2{tag}"] = nc.dram_tensor(f"w2{tag}", [MT2, P, KT2 * P], bf16,
                                          kind="ExternalInput").ap()
        dram[f"b1{tag}"] = nc.dram_tensor(f"b1{tag}", [P, MT1], f32,
                                          kind="ExternalInput").ap()
        dram[f"y{tag}"] = nc.dram_tensor(f"y{tag}", [MT2, P, cap], f32,
                                         kind="ExternalOutput").ap()

    with tile.TileContext(nc) as tc:
        with (
            tc.tile_pool(name="const", bufs=1) as const,
            tc.tile_pool(name="xp", bufs=1) as xp,
            tc.tile_pool(name="hp", bufs=1) as hp,
            tc.tile_pool(name="w1p", bufs=MT1) as w1p,
            tc.tile_pool(name="w2p", bufs=3) as w2p,
            tc.tile_pool(name="psp", bufs=8, space="PSUM") as psp,
            tc.tile_pool(name="op", bufs=6) as op,
        ):
            chA = _chunks(capA)
            chB = _chunks(capB)

            # ---- prologue DMAs (sync queue: x + biases; gpsimd: weights) ----
            xa = xp.tile([P, KT1, capA], bf16, name="xa")
            xb = xp.tile([P, KT1, capB], bf16, name="xb")

            # x chunks spread across the sync/vector/scalar DMA queues so they
            # arrive in parallel right after queue startup (~6us); vector and
            # scalar have no compute until well after these transfers finish.
            def xdma(queue, x_sb, tag, cs, cw):
                return queue.dma_start(
                    x_sb[:, :, cs:cs + cw],
                    dram[f"xg{tag}"][:, 8 * cs:8 * (cs + cw)]
                    .rearrange("p (k c) -> p k c", k=KT1))

            qa = [nc.sync, nc.scalar, nc.sync]
            xdma(qa[0], xa, "a", *chA[0])
            b1s = {}
            for tag in ("a", "b"):
                b1s[tag] = const.tile([P, MT1], f32, name=f"b1{tag}")
                nc.sync.dma_start(b1s[tag][:], dram[f"b1{tag}"][:, :])
            for ci, (cs, cw) in enumerate(chA[1:], 1):
                xdma(qa[ci % len(qa)], xa, "a", cs, cw)
            # xb is issued on the sync queue but explicitly held back until
            # phase 1 is underway: the first ~15us are HBM-contended across
            # all 8 cores (weights + x everywhere), and xb isn't needed until
            # the second half of the kernel.
            xb_dmas = [xdma(nc.sync, xb, "b", cs, cw) for cs, cw in chB]

            # PE warm-up: the tensor engine clock ramps (0.65 -> 1.2 ->
            # 2.4 GHz over ~3us of sustained activity). Dummy matmuls that
            # depend only on an on-chip memset keep the PE busy before the
            # first x/w DMAs land, so real work starts at full clock.
            warm = const.tile([P, 256], bf16, name="warm")
            nc.vector.memset(warm[:], 0)
            wps = psp.tile([P, 256], f32, tag="ps", name="warmps")
            for _ in range(12):
                nc.tensor.matmul(wps[:], warm[:, :128], warm[:],
                                 start=True, stop=True)

            for tag, cap, chunks, x_sb in (("a", capA, chA, xa),
                                           ("b", capB, chB, xb)):
                hT = hp.tile([P, MT1, cap], bf16, name=f"h{tag}")
                # ---- phase 1: hT = gelu(W1h.T @ xT + b1h) ----
                # chunk-OUTER: the whole first (smallest) chunk is computed
                # across all 16 m-tiles before the later chunks are touched,
                # so the PE only ever depends on x chunks that have already
                # arrived (the full x transfer is HBM-contended at startup
                # when all 8 cores pull their streams at once). All 16 w1
                # tiles stay resident (32KB/partition in bf16).
                w1ts = []
                for m in range(MT1):
                    w1m = w1p.tile([P, KT1, P], bf16, tag="w1")
                    nc.gpsimd.dma_start(
                        w1m[:], dram[f"w1{tag}"][m].rearrange("p (k q) -> p k q", k=KT1))
                    w1ts.append(w1m)
                for ci, (cs, cw) in enumerate(chunks):
                    for m in range(MT1):
                        ps = psp.tile([P, cw], f32, tag="ps", name=f"ps{ci}")
                        for kt in range(KT1):
                            nc.tensor.matmul(
                                ps[:],
                                w1ts[m][:, kt, :],
                                x_sb[:, kt, cs:cs + cw],
                                start=(kt == 0),
                                stop=(kt == KT1 - 1),
                            )
                        act = nc.scalar.activation(
                            hT[:, m, cs:cs + cw], ps[:], gelu,
                            bias=b1s[tag][:, m:m + 1],
                        )
                        if tag == "a" and ci == 0 and m == 6 and xb_dmas:
                            for d in xb_dmas:
                                add_dep_helper(d.ins, act.ins, sync=False,
                                               reason="hold xb past startup storm")
                            xb_dmas = []
                # ---- phase 2: y = W2h.T @ hT + b2 (partial; host sums halves)
                for m in range(MT2):
                    w2m = w2p.tile([P, KT2, P], bf16, tag="w2")
                    nc.gpsimd.dma_start(
                        w2m[:], dram[f"w2{tag}"][m].rearrange("p (k q) -> p k q", k=KT2))
                    mchunks = chunks
                    if tag == "b" and m == MT2 - 1:
                        # final m-tile: big chunks first, then a tiny last
                        # chunk so the closing evacuate+store tail is short
                        parts = [c for _, c in _chunks(cap - 96, reverse=True)]
                        parts.append(96)
                        mchunks, off = [], 0
                        for c in parts:
                            mchunks.append((off, c))
                            off += c
                    for ci, (cs, cw) in enumerate(mchunks):
                        ps = psp.tile([P, cw], f32, tag="ps", name=f"ps{ci}")
                        for kq in range(KT2):
                            nc.tensor.matmul(
                                ps[:],
                                w2m[:, kq, :],
                                hT[:, kq, cs:cs + cw],
                                start=(kq == 0),
                                stop=(kq == KT2 - 1),
                            )
                        # evacuate psum and store; b2 is added on the host
                        # during the combine step. The final phase's stores
                        # alternate between the sync and gpsimd DMA queues so
                        # the kernel tail isn't one serialized store queue
                        # (the gpsimd queue is done with weights by then).
                        ot = op.tile([P, cw], f32, tag="o", name=f"o{ci}")
                        nc.vector.tensor_copy(ot[:], ps[:])
                        if tag == "b":
                            # spread the final phase's stores over three DMA
                            # queues (all idle by then) to shorten the tail
                            q = [nc.sync, nc.gpsimd, nc.scalar][(m + ci) % 3]
                        else:
                            q = nc.sync
                        q.dma_start(dram[f"y{tag}"][m, :, cs:cs + cw], ot[:])

    nc.compile()
    return nc


def _pad16(n):
    return max(256, -(-n // 16) * 16)


def kernel(x, Wg, bg, W1, b1, W2, b2):
    global LAST_RESULT
    _ensure_axon_hooks()
    import ml_dtypes
    from concourse.bass_utils import run_bass_kernel_spmd

    bf = ml_dtypes.bfloat16
    x = np.ascontiguousarray(np.asarray(x, dtype=np.float32))
    Wg = np.asarray(Wg, dtype=np.float32)
    bg = np.asarray(bg, dtype=np.float32)
    W1 = np.asarray(W1, dtype=np.float32)
    b1 = np.asarray(b1, dtype=np.float32)
    W2 = np.asarray(W2, dtype=np.float32)
    b2 = np.asarray(b2, dtype=np.float32)

    B, S, D = x.shape
    T = B * S
    xf = x.reshape(T, D)

    top_idx, top_w = _route(xf, Wg, bg)

    tok_idx, tok_w = [], []
    for e in range(NUM_EXPERTS):
        sel = top_idx == e
        rows = np.nonzero(sel.any(axis=1))[0]
        tok_idx.append(rows)
        tok_w.append((top_w * sel).sum(axis=1)[rows].astype(np.float32))

    loads = [len(r) for r in tok_idx]
    order = np.argsort(-np.asarray(loads), kind="stable")
    # pair hottest with coldest: pair p = (order[p], order[7-p])
    pairs = [(int(order[p]), int(order[NUM_EXPERTS - 1 - p]))
             for p in range(NUM_EXPERTS // 2)]
    capA = _pad16(max(loads[a] for a, _ in pairs))
    capB = _pad16(max(loads[b] for _, b in pairs))

    nc = _build_device_program(capA, capB)

    def prep_x(e, cap):
        idx_pad = np.zeros(cap, dtype=np.int64)
        idx_pad[:loads[e]] = tok_idx[e]
        xg = xf[idx_pad].T.reshape(KT1, P, cap).transpose(1, 0, 2).astype(bf)
        # chunk-major: one contiguous [P, KT1*cw] block per chunk
        blocks = [np.ascontiguousarray(xg[:, :, cs:cs + cw]).reshape(P, -1)
                  for cs, cw in _chunks(cap)]
        return np.ascontiguousarray(np.concatenate(blocks, axis=1))

    def prep_w(e, half):
        fh = slice(half * FH, (half + 1) * FH)
        w1t = np.ascontiguousarray(
            W1[e][:, fh].reshape(KT1, P, MT1, P).transpose(2, 1, 0, 3)
            .reshape(MT1, P, KT1 * P).astype(bf))
        w2t = np.ascontiguousarray(
            W2[e][fh, :].reshape(KT2, P, MT2, P).transpose(2, 1, 0, 3)
            .reshape(MT2, P, KT2 * P).astype(bf))
        b1t = np.ascontiguousarray(b1[e][fh].reshape(MT1, P).T)
        return w1t, w2t, b1t

    in_maps = []
    xg_cache = {}
    for p, (ea, eb) in enumerate(pairs):
        xg_cache[ea] = prep_x(ea, capA)
        xg_cache[eb] = prep_x(eb, capB)
        for half in range(2):
            w1a, w2a, b1a = prep_w(ea, half)
            w1b, w2b, b1b = prep_w(eb, half)
            in_maps.append({
                "xga": xg_cache[ea], "w1a": w1a, "w2a": w2a, "b1a": b1a,
                "xgb": xg_cache[eb], "w1b": w1b, "w2b": w2b, "b1b": b1b,
            })

    import os
    trace_cores = None
    if os.environ.get("MOE_TRACE_ALL"):
        trace_cores = list(range(NUM_EXPERTS))
    res = run_bass_kernel_spmd(nc, in_maps, core_ids=list(range(NUM_EXPERTS)),
                               trace_cores=trace_cores)
    LAST_RESULT = res

    out = np.zeros((T, D), dtype=np.float32)
    for p, (ea, eb) in enumerate(pairs):
        for e, key, cap in ((ea, "ya", capA), (eb, "yb", capB)):
            n_e = loads[e]
            if n_e == 0:
                continue
            yT = (res.results[2 * p][key].astype(np.float32)
                  + res.results[2 * p + 1][key].astype(np.float32)).reshape(D, cap)
            out[tok_idx[e]] += tok_w[e][:, None] * (yT[:, :n_e].T + b2[e])
    return out.reshape(B, S, D)


# revision 32
# speedup vs baseline: 1.0294x; 1.0294x over previous
"""MoE layer (8 experts, top-2) on 8 Trainium2 NeuronCores.

Strategy (v2): hidden-dim-split expert parallelism in bf16.
  - Host computes gating + top-2 routing (mirrors the reference ops).
  - Experts are sorted by token load and paired hot-with-cold; pair p is
    assigned to cores (2p, 2p+1), each core computing one HALF of the FFN
    hidden dim (2048 of 4096) for BOTH experts of the pair. This balances
    the per-core matmul work to ~(L_hot+L_cold)/2 token-columns regardless
    of routing skew, while keeping per-core weight traffic identical to
    one full expert (weights stream from HBM exactly once).
  - All matmul operands are bf16 (PE rate is identical to fp32r, but DMA
    bytes and SBUF footprint halve); PSUM accumulation is fp32 and the
    partial outputs return as fp32.
  - Host sums the two half partials, applies combine weights, and
    scatter-adds into token order.

Device layout: activations are transposed ([feature, token]); x lives in
SBUF as [128, 8, cap] (k-subtile middle), h as [128, 16, cap] bf16 which
fits residently, so phase 2 needs no hidden-dim quartering and y needs no
multi-pass accumulation: each phase-2 psum group covers the full 2048
contraction and evacuates straight to the output DMA.
"""

import numpy as np

N_EMBED = 1024
FFN_HIDDEN = 4096
NUM_EXPERTS = 8
TOP_K = 2
P = 128
KT1 = N_EMBED // P          # 8  k-tiles, phase 1
FH = FFN_HIDDEN // 2        # 2048 hidden features per core (half)
MT1 = FH // P               # 16 m-tiles, phase 1 (half hidden)
KT2 = FH // P               # 16 k-tiles, phase 2
MT2 = N_EMBED // P          # 8  m-tiles, phase 2

LAST_RESULT = None          # BassKernelResults of the most recent run


def _ensure_axon_hooks():
    """Make `antenv.axon_hooks` importable so BASS_TRACE=1 degrades
    gracefully instead of crashing when the image lacks the module."""
    try:
        import antenv.axon_hooks  # noqa: F401
        return
    except ImportError:
        pass
    import sys
    import types

    m = types.ModuleType("antenv.axon_hooks")
    m._hook = None
    m.set_axon_ntff_profile_hook = lambda h: setattr(m, "_hook", h)
    m.get_axon_ntff_profile_hook = lambda: m._hook
    sys.modules["antenv.axon_hooks"] = m
    try:
        from trn_agent_boot.trn_boot import _ntff_profile_via_ctypes

        m.set_axon_ntff_profile_hook(_ntff_profile_via_ctypes("/opt/axon/libaxon_pjrt.so"))
    except Exception:
        pass


def _route(x2d, Wg, bg):
    """Top-2 gating. Mirrors the reference (jax softmax + lax.top_k) so the
    selected experts match it exactly; numpy fallback is numerically
    equivalent up to fp32 rounding."""
    try:
        import jax
        import jax.numpy as jnp

        gate = jax.nn.softmax(jnp.asarray(x2d) @ jnp.asarray(Wg) + jnp.asarray(bg), axis=-1)
        scores, idx = jax.lax.top_k(gate, TOP_K)
        scores = np.asarray(scores, dtype=np.float32)
        idx = np.asarray(idx)
    except Exception:
        logits = x2d @ Wg + bg
        m = logits.max(-1, keepdims=True)
        e = np.exp(logits - m)
        p = e / e.sum(-1, keepdims=True)
        order = np.argsort(-p, axis=-1, kind="stable")
        idx = order[:, :TOP_K]
        scores = np.take_along_axis(p, idx, axis=-1)
    scores = scores / (scores.sum(-1, keepdims=True) + 1e-8)
    return idx.astype(np.int64), scores.astype(np.float32)


def _chunks(width, reverse=False):
    """Split a token capacity into matmul free-dim chunks of at most 512
    (one PSUM bank of fp32). Smallest chunk first so the kernel's very
    first psum group is short; reverse=True puts it last (for the tail)."""
    out, rem = [], width
    while rem > 0:
        c = min(rem, 512)
        out.append(c)
        rem -= c
    out.sort(reverse=reverse)
    res, off = [], 0
    for c in out:
        res.append((off, c))
        off += c
    return res


def _chunksA(cap):
    """A-side chunk layout: first chunk pinned to 256 columns (see
    _build_device_program), remainder split ascending."""
    return [(0, 256)] + [(cs + 256, cw) for cs, cw in _chunks(cap - 256)]


def _build_device_program(capA, capB):
    import concourse.tile as tile
    from concourse import bacc, mybir
    from concourse.tile_rust import add_dep_helper

    f32 = mybir.dt.float32
    bf16 = mybir.dt.bfloat16
    gelu = mybir.ActivationFunctionType.Gelu_apprx_tanh

    nc = bacc.Bacc("TRN2", target_bir_lowering=False, debug=False,
                   num_devices=NUM_EXPERTS)

    dram = {}
    for tag, cap in (("a", capA), ("b", capB)):
        # x is stored chunk-major: contiguous [P, KT1*cw] blocks per chunk,
        # so every chunk DMA moves contiguous per-partition runs at full
        # bandwidth (a strided whole-tensor DMA measured ~60% slower and
        # stalled the first phase for ~12us).
        dram[f"xg{tag}"] = nc.dram_tensor(f"xg{tag}", [P, KT1 * cap], bf16,
                                          kind="ExternalInput").ap()
        dram[f"w1{tag}"] = nc.dram_tensor(f"w1{tag}", [MT1, P, KT1 * P], bf16,
                                          kind="ExternalInput").ap()
        dram[f"w2{tag}"] = nc.dram_tensor(f"w2{tag}", [MT2, P, KT2 * P], bf16,
                                          kind="ExternalInput").ap()
        dram[f"b1{tag}"] = nc.dram_tensor(f"b1{tag}", [P, MT1], f32,
                                          kind="ExternalInput").ap()
        dram[f"y{tag}"] = nc.dram_tensor(f"y{tag}", [MT2, P, cap], f32,
                                         kind="ExternalOutput").ap()

    # A-side chunks: first chunk pinned to 256 columns. Smaller first chunks
    # make pass-0 psum groups so short that the PE outruns the HBM-contended
    # w1 stream; each resulting idle gap resets the PE clock ramp and the
    # whole pass crawls at the mid p-state.
    chA = _chunksA(capA)
    chB = _chunks(capB)

    with tile.TileContext(nc) as tc:
        with (
            tc.tile_pool(name="const", bufs=1) as const,
            tc.tile_pool(name="xp", bufs=1) as xp,
            tc.tile_pool(name="hp", bufs=1) as hp,
            tc.tile_pool(name="w1p", bufs=MT1) as w1p,
            tc.tile_pool(name="w2p", bufs=3) as w2p,
            tc.tile_pool(name="psp", bufs=8, space="PSUM") as psp,
            tc.tile_pool(name="op", bufs=6) as op,
        ):
            # ---- prologue DMAs (sync queue: x + biases; gpsimd: weights) ----
            xa = xp.tile([P, KT1, capA], bf16, name="xa")
            xb = xp.tile([P, KT1, capB], bf16, name="xb")

            # x chunks spread across the sync/vector/scalar DMA queues so they
            # arrive in parallel right after queue startup (~6us); vector and
            # scalar have no compute until well after these transfers finish.
            def xdma(queue, x_sb, tag, cs, cw):
                return queue.dma_start(
                    x_sb[:, :, cs:cs + cw],
                    dram[f"xg{tag}"][:, 8 * cs:8 * (cs + cw)]
                    .rearrange("p (k c) -> p k c", k=KT1))

            qa = [nc.sync, nc.scalar, nc.sync]
            xdma(qa[0], xa, "a", *chA[0])
            b1s = {}
            for tag in ("a", "b"):
                b1s[tag] = const.tile([P, MT1], f32, name=f"b1{tag}")
                nc.sync.dma_start(b1s[tag][:], dram[f"b1{tag}"][:, :])
            for ci, (cs, cw) in enumerate(chA[1:], 1):
                xdma(qa[ci % len(qa)], xa, "a", cs, cw)
            # xb is issued on the sync queue but explicitly held back until
            # phase 1 is underway: the first ~15us are HBM-contended across
            # all 8 cores (weights + x everywhere), and xb isn't needed until
            # the second half of the kernel.
            xb_dmas = [xdma(nc.sync, xb, "b", cs, cw) for cs, cw in chB]

            # PE warm-up: the tensor engine clock ramps (0.65 -> 1.2 ->
            # 2.4 GHz over ~3us of sustained activity). Dummy matmuls that
            # depend only on an on-chip memset keep the PE busy before the
            # first x/w DMAs land, so real work starts at full clock.
            warm = const.tile([P, 256], bf16, name="warm")
            nc.vector.memset(warm[:], 0)
            wps = psp.tile([P, 256], f32, tag="ps", name="warmps")
            for _ in range(30):
                nc.tensor.matmul(wps[:], warm[:, :128], warm[:],
                                 start=True, stop=True)

            for tag, cap, chunks, x_sb in (("a", capA, chA, xa),
                                           ("b", capB, chB, xb)):
                hT = hp.tile([P, MT1, cap], bf16, name=f"h{tag}")
                # ---- phase 1: hT = gelu(W1h.T @ xT + b1h) ----
                # chunk-OUTER: the whole first (smallest) chunk is computed
                # across all 16 m-tiles before the later chunks are touched,
                # so the PE only ever depends on x chunks that have already
                # arrived (the full x transfer is HBM-contended at startup
                # when all 8 cores pull their streams at once). All 16 w1
                # tiles stay resident (32KB/partition in bf16).
                w1ts = []
                for m in range(MT1):
                    w1m = w1p.tile([P, KT1, P], bf16, tag="w1")
                    nc.gpsimd.dma_start(
                        w1m[:], dram[f"w1{tag}"][m].rearrange("p (k q) -> p k q", k=KT1))
                    w1ts.append(w1m)
                for ci, (cs, cw) in enumerate(chunks):
                    for m in range(MT1):
                        ps = psp.tile([P, cw], f32, tag="ps", name=f"ps{ci}")
                        for kt in range(KT1):
                            nc.tensor.matmul(
                                ps[:],
                                w1ts[m][:, kt, :],
                                x_sb[:, kt, cs:cs + cw],
                                start=(kt == 0),
                                stop=(kt == KT1 - 1),
                            )
                        act = nc.scalar.activation(
                            hT[:, m, cs:cs + cw], ps[:], gelu,
                            bias=b1s[tag][:, m:m + 1],
                        )
                        if tag == "a" and ci == 0 and m == 6 and xb_dmas:
                            for d in xb_dmas:
                                add_dep_helper(d.ins, act.ins, sync=False,
                                               reason="hold xb past startup storm")
                            xb_dmas = []
                # ---- phase 2: y = W2h.T @ hT + b2 (partial; host sums halves)
                for m in range(MT2):
                    w2m = w2p.tile([P, KT2, P], bf16, tag="w2")
                    nc.gpsimd.dma_start(
                        w2m[:], dram[f"w2{tag}"][m].rearrange("p (k q) -> p k q", k=KT2))
                    mchunks = chunks
                    if tag == "b" and m == MT2 - 1:
                        # final m-tile: big chunks first, then a tiny last
                        # chunk so the closing evacuate+store tail is short
                        parts = [c for _, c in _chunks(cap - 96, reverse=True)]
                        parts.append(96)
                        mchunks, off = [], 0
                        for c in parts:
                            mchunks.append((off, c))
                            off += c
                    for ci, (cs, cw) in enumerate(mchunks):
                        ps = psp.tile([P, cw], f32, tag="ps", name=f"ps{ci}")
                        for kq in range(KT2):
                            nc.tensor.matmul(
                                ps[:],
                                w2m[:, kq, :],
                                hT[:, kq, cs:cs + cw],
                                start=(kq == 0),
                                stop=(kq == KT2 - 1),
                            )
                        # evacuate psum and store; b2 is added on the host
                        # during the combine step. The final phase's stores
                        # alternate between the sync and gpsimd DMA queues so
                        # the kernel tail isn't one serialized store queue
                        # (the gpsimd queue is done with weights by then).
                        ot = op.tile([P, cw], f32, tag="o", name=f"o{ci}")
                        nc.vector.tensor_copy(ot[:], ps[:])
                        if tag == "b":
                            # spread the final phase's stores over three DMA
                            # queues (all idle by then) to shorten the tail
                            q = [nc.sync, nc.gpsimd, nc.scalar][(m + ci) % 3]
                        else:
                            q = nc.sync
                        q.dma_start(dram[f"y{tag}"][m, :, cs:cs + cw], ot[:])

    nc.compile()
    return nc


def _pad16(n):
    return max(256, -(-n // 16) * 16)


def kernel(x, Wg, bg, W1, b1, W2, b2):
    global LAST_RESULT
    _ensure_axon_hooks()
    import ml_dtypes
    from concourse.bass_utils import run_bass_kernel_spmd

    bf = ml_dtypes.bfloat16
    x = np.ascontiguousarray(np.asarray(x, dtype=np.float32))
    Wg = np.asarray(Wg, dtype=np.float32)
    bg = np.asarray(bg, dtype=np.float32)
    W1 = np.asarray(W1, dtype=np.float32)
    b1 = np.asarray(b1, dtype=np.float32)
    W2 = np.asarray(W2, dtype=np.float32)
    b2 = np.asarray(b2, dtype=np.float32)

    B, S, D = x.shape
    T = B * S
    xf = x.reshape(T, D)

    top_idx, top_w = _route(xf, Wg, bg)

    tok_idx, tok_w = [], []
    for e in range(NUM_EXPERTS):
        sel = top_idx == e
        rows = np.nonzero(sel.any(axis=1))[0]
        tok_idx.append(rows)
        tok_w.append((top_w * sel).sum(axis=1)[rows].astype(np.float32))

    loads = [len(r) for r in tok_idx]
    order = np.argsort(-np.asarray(loads), kind="stable")
    # pair hottest with coldest: pair p = (order[p], order[7-p])
    pairs = [(int(order[p]), int(order[NUM_EXPERTS - 1 - p]))
             for p in range(NUM_EXPERTS // 2)]
    capA = _pad16(max(loads[a] for a, _ in pairs))
    capB = _pad16(max(loads[b] for _, b in pairs))

    nc = _build_device_program(capA, capB)

    def prep_x(e, cap, chunks):
        idx_pad = np.zeros(cap, dtype=np.int64)
        idx_pad[:loads[e]] = tok_idx[e]
        xg = xf[idx_pad].T.reshape(KT1, P, cap).transpose(1, 0, 2).astype(bf)
        # chunk-major: one contiguous [P, KT1*cw] block per chunk, with the
        # same chunk boundaries the device program uses
        blocks = [np.ascontiguousarray(xg[:, :, cs:cs + cw]).reshape(P, -1)
                  for cs, cw in chunks]
        return np.ascontiguousarray(np.concatenate(blocks, axis=1))

    def prep_w(e, half):
        fh = slice(half * FH, (half + 1) * FH)
        w1t = np.ascontiguousarray(
            W1[e][:, fh].reshape(KT1, P, MT1, P).transpose(2, 1, 0, 3)
            .reshape(MT1, P, KT1 * P).astype(bf))
        w2t = np.ascontiguousarray(
            W2[e][fh, :].reshape(KT2, P, MT2, P).transpose(2, 1, 0, 3)
            .reshape(MT2, P, KT2 * P).astype(bf))
        b1t = np.ascontiguousarray(b1[e][fh].reshape(MT1, P).T)
        return w1t, w2t, b1t

    in_maps = []
    xg_cache = {}
    for p, (ea, eb) in enumerate(pairs):
        xg_cache[ea] = prep_x(ea, capA, _chunksA(capA))
        xg_cache[eb] = prep_x(eb, capB, _chunks(capB))
        for half in range(2):
            w1a, w2a, b1a = prep_w(ea, half)
            w1b, w2b, b1b = prep_w(eb, half)
            in_maps.append({
                "xga": xg_cache[ea], "w1a": w1a, "w2a": w2a, "b1a": b1a,
                "xgb": xg_cache[eb], "w1b": w1b, "w2b": w2b, "b1b": b1b,
            })

    import os
    trace_cores = None
    if os.environ.get("MOE_TRACE_ALL"):
        trace_cores = list(range(NUM_EXPERTS))
    res = run_bass_kernel_spmd(nc, in_maps, core_ids=list(range(NUM_EXPERTS)),
                               trace_cores=trace_cores)
    LAST_RESULT = res

    out = np.zeros((T, D), dtype=np.float32)
    for p, (ea, eb) in enumerate(pairs):
        for e, key, cap in ((ea, "ya", capA), (eb, "yb", capB)):
            n_e = loads[e]
            if n_e == 0:
                continue
            yT = (res.results[2 * p][key].astype(np.float32)
                  + res.results[2 * p + 1][key].astype(np.float32)).reshape(D, cap)
            out[tok_idx[e]] += tok_w[e][:, None] * (yT[:, :n_e].T + b2[e])
    return out.reshape(B, S, D)


# revision 35
# speedup vs baseline: 1.0305x; 1.0011x over previous
"""MoE layer (8 experts, top-2) on 8 Trainium2 NeuronCores.

Strategy (v2): hidden-dim-split expert parallelism in bf16.
  - Host computes gating + top-2 routing (mirrors the reference ops).
  - Experts are sorted by token load and paired hot-with-cold; pair p is
    assigned to cores (2p, 2p+1), each core computing one HALF of the FFN
    hidden dim (2048 of 4096) for BOTH experts of the pair. This balances
    the per-core matmul work to ~(L_hot+L_cold)/2 token-columns regardless
    of routing skew, while keeping per-core weight traffic identical to
    one full expert (weights stream from HBM exactly once).
  - All matmul operands are bf16 (PE rate is identical to fp32r, but DMA
    bytes and SBUF footprint halve); PSUM accumulation is fp32 and the
    partial outputs return as fp32.
  - Host sums the two half partials, applies combine weights, and
    scatter-adds into token order.

Device layout: activations are transposed ([feature, token]); x lives in
SBUF as [128, 8, cap] (k-subtile middle), h as [128, 16, cap] bf16 which
fits residently, so phase 2 needs no hidden-dim quartering and y needs no
multi-pass accumulation: each phase-2 psum group covers the full 2048
contraction and evacuates straight to the output DMA.
"""

import numpy as np

N_EMBED = 1024
FFN_HIDDEN = 4096
NUM_EXPERTS = 8
TOP_K = 2
P = 128
KT1 = N_EMBED // P          # 8  k-tiles, phase 1
FH = FFN_HIDDEN // 2        # 2048 hidden features per core (half)
MT1 = FH // P               # 16 m-tiles, phase 1 (half hidden)
KT2 = FH // P               # 16 k-tiles, phase 2
MT2 = N_EMBED // P          # 8  m-tiles, phase 2

LAST_RESULT = None          # BassKernelResults of the most recent run


def _ensure_axon_hooks():
    """Make `antenv.axon_hooks` importable so BASS_TRACE=1 degrades
    gracefully instead of crashing when the image lacks the module."""
    try:
        import antenv.axon_hooks  # noqa: F401
        return
    except ImportError:
        pass
    import sys
    import types

    m = types.ModuleType("antenv.axon_hooks")
    m._hook = None
    m.set_axon_ntff_profile_hook = lambda h: setattr(m, "_hook", h)
    m.get_axon_ntff_profile_hook = lambda: m._hook
    sys.modules["antenv.axon_hooks"] = m
    try:
        from trn_agent_boot.trn_boot import _ntff_profile_via_ctypes

        m.set_axon_ntff_profile_hook(_ntff_profile_via_ctypes("/opt/axon/libaxon_pjrt.so"))
    except Exception:
        pass


def _route(x2d, Wg, bg):
    """Top-2 gating. Mirrors the reference (jax softmax + lax.top_k) so the
    selected experts match it exactly; numpy fallback is numerically
    equivalent up to fp32 rounding."""
    try:
        import jax
        import jax.numpy as jnp

        gate = jax.nn.softmax(jnp.asarray(x2d) @ jnp.asarray(Wg) + jnp.asarray(bg), axis=-1)
        scores, idx = jax.lax.top_k(gate, TOP_K)
        scores = np.asarray(scores, dtype=np.float32)
        idx = np.asarray(idx)
    except Exception:
        logits = x2d @ Wg + bg
        m = logits.max(-1, keepdims=True)
        e = np.exp(logits - m)
        p = e / e.sum(-1, keepdims=True)
        order = np.argsort(-p, axis=-1, kind="stable")
        idx = order[:, :TOP_K]
        scores = np.take_along_axis(p, idx, axis=-1)
    scores = scores / (scores.sum(-1, keepdims=True) + 1e-8)
    return idx.astype(np.int64), scores.astype(np.float32)


def _chunks(width, reverse=False):
    """Split a token capacity into matmul free-dim chunks of at most 512
    (one PSUM bank of fp32). Smallest chunk first so the kernel's very
    first psum group is short; reverse=True puts it last (for the tail)."""
    out, rem = [], width
    while rem > 0:
        c = min(rem, 512)
        out.append(c)
        rem -= c
    out.sort(reverse=reverse)
    res, off = [], 0
    for c in out:
        res.append((off, c))
        off += c
    return res


def _chunksA(cap):
    """A-side chunk layout: first chunk pinned to 256 columns (see
    _build_device_program), remainder split ascending."""
    return [(0, 256)] + [(cs + 256, cw) for cs, cw in _chunks(cap - 256)]


def _build_device_program(capA, capB):
    import concourse.tile as tile
    from concourse import bacc, mybir
    from concourse.tile_rust import add_dep_helper

    f32 = mybir.dt.float32
    bf16 = mybir.dt.bfloat16
    gelu = mybir.ActivationFunctionType.Gelu_apprx_tanh

    nc = bacc.Bacc("TRN2", target_bir_lowering=False, debug=False,
                   num_devices=NUM_EXPERTS)

    dram = {}
    for tag, cap in (("a", capA), ("b", capB)):
        # x is stored chunk-major: contiguous [P, KT1*cw] blocks per chunk,
        # so every chunk DMA moves contiguous per-partition runs at full
        # bandwidth (a strided whole-tensor DMA measured ~60% slower and
        # stalled the first phase for ~12us).
        dram[f"xg{tag}"] = nc.dram_tensor(f"xg{tag}", [P, KT1 * cap], bf16,
                                          kind="ExternalInput").ap()
        dram[f"w1{tag}"] = nc.dram_tensor(f"w1{tag}", [MT1, P, KT1 * P], bf16,
                                          kind="ExternalInput").ap()
        dram[f"w2{tag}"] = nc.dram_tensor(f"w2{tag}", [MT2, P, KT2 * P], bf16,
                                          kind="ExternalInput").ap()
        dram[f"b1{tag}"] = nc.dram_tensor(f"b1{tag}", [P, MT1], f32,
                                          kind="ExternalInput").ap()
        dram[f"y{tag}"] = nc.dram_tensor(f"y{tag}", [MT2, P, cap], f32,
                                         kind="ExternalOutput").ap()

    # A-side chunks: first chunk pinned to 256 columns. Smaller first chunks
    # make pass-0 psum groups so short that the PE outruns the HBM-contended
    # w1 stream; each resulting idle gap resets the PE clock ramp and the
    # whole pass crawls at the mid p-state.
    chA = _chunksA(capA)
    chB = _chunks(capB)

    with tile.TileContext(nc) as tc:
        with (
            tc.tile_pool(name="const", bufs=1) as const,
            tc.tile_pool(name="xp", bufs=1) as xp,
            tc.tile_pool(name="hp", bufs=1) as hp,
            tc.tile_pool(name="w1p", bufs=MT1) as w1p,
            tc.tile_pool(name="w2p", bufs=3) as w2p,
            tc.tile_pool(name="psp", bufs=8, space="PSUM") as psp,
            tc.tile_pool(name="op", bufs=6) as op,
        ):
            # ---- prologue DMAs (sync queue: x + biases; gpsimd: weights) ----
            xa = xp.tile([P, KT1, capA], bf16, name="xa")
            xb = xp.tile([P, KT1, capB], bf16, name="xb")

            # x chunks spread across the sync/vector/scalar DMA queues so they
            # arrive in parallel right after queue startup (~6us); vector and
            # scalar have no compute until well after these transfers finish.
            def xdma(queue, x_sb, tag, cs, cw):
                return queue.dma_start(
                    x_sb[:, :, cs:cs + cw],
                    dram[f"xg{tag}"][:, 8 * cs:8 * (cs + cw)]
                    .rearrange("p (k c) -> p k c", k=KT1))

            qa = [nc.sync, nc.scalar, nc.sync]
            xdma(qa[0], xa, "a", *chA[0])
            b1s = {}
            for tag in ("a", "b"):
                b1s[tag] = const.tile([P, MT1], f32, name=f"b1{tag}")
                nc.sync.dma_start(b1s[tag][:], dram[f"b1{tag}"][:, :])
            held_dmas = []
            for ci, (cs, cw) in enumerate(chA[1:], 1):
                d = xdma(qa[ci % len(qa)], xa, "a", cs, cw)
                if ci >= 2:
                    # the third A-chunk isn't consumed until pass 2 (~+35us);
                    # keep it out of the contended startup window as well
                    held_dmas.append(d)
            # xb is issued on the sync queue but explicitly held back until
            # phase 1 is underway: the first ~15us are HBM-contended across
            # all 8 cores (weights + x everywhere), and xb isn't needed until
            # the second half of the kernel.
            xb_dmas = [xdma(nc.sync, xb, "b", cs, cw) for cs, cw in chB]

            # PE warm-up: the tensor engine clock ramps (0.65 -> 1.2 ->
            # 2.4 GHz over ~3us of sustained activity). Dummy matmuls that
            # depend only on an on-chip memset keep the PE busy before the
            # first x/w DMAs land, so real work starts at full clock.
            warm = const.tile([P, 256], bf16, name="warm")
            nc.vector.memset(warm[:], 0)
            wps = psp.tile([P, 256], f32, tag="ps", name="warmps")
            for _ in range(26):
                nc.tensor.matmul(wps[:], warm[:, :128], warm[:],
                                 start=True, stop=True)

            for tag, cap, chunks, x_sb in (("a", capA, chA, xa),
                                           ("b", capB, chB, xb)):
                hT = hp.tile([P, MT1, cap], bf16, name=f"h{tag}")
                # ---- phase 1: hT = gelu(W1h.T @ xT + b1h) ----
                # chunk-OUTER: the whole first (smallest) chunk is computed
                # across all 16 m-tiles before the later chunks are touched,
                # so the PE only ever depends on x chunks that have already
                # arrived (the full x transfer is HBM-contended at startup
                # when all 8 cores pull their streams at once). All 16 w1
                # tiles stay resident (32KB/partition in bf16).
                w1ts = []
                for m in range(MT1):
                    w1m = w1p.tile([P, KT1, P], bf16, tag="w1")
                    nc.gpsimd.dma_start(
                        w1m[:], dram[f"w1{tag}"][m].rearrange("p (k q) -> p k q", k=KT1))
                    w1ts.append(w1m)
                for ci, (cs, cw) in enumerate(chunks):
                    for m in range(MT1):
                        ps = psp.tile([P, cw], f32, tag="ps", name=f"ps{ci}")
                        for kt in range(KT1):
                            nc.tensor.matmul(
                                ps[:],
                                w1ts[m][:, kt, :],
                                x_sb[:, kt, cs:cs + cw],
                                start=(kt == 0),
                                stop=(kt == KT1 - 1),
                            )
                        act = nc.scalar.activation(
                            hT[:, m, cs:cs + cw], ps[:], gelu,
                            bias=b1s[tag][:, m:m + 1],
                        )
                        if tag == "a" and ci == 0 and m == 2 and held_dmas:
                            for d in held_dmas:
                                add_dep_helper(d.ins, act.ins, sync=False,
                                               reason="hold late x past startup storm")
                            held_dmas = []
                        if tag == "a" and ci == 0 and m == 6 and xb_dmas:
                            for d in xb_dmas:
                                add_dep_helper(d.ins, act.ins, sync=False,
                                               reason="hold xb past startup storm")
                            xb_dmas = []
                # ---- phase 2: y = W2h.T @ hT + b2 (partial; host sums halves)
                for m in range(MT2):
                    w2m = w2p.tile([P, KT2, P], bf16, tag="w2")
                    nc.gpsimd.dma_start(
                        w2m[:], dram[f"w2{tag}"][m].rearrange("p (k q) -> p k q", k=KT2))
                    mchunks = chunks
                    if tag == "b" and m == MT2 - 1:
                        # final m-tile: big chunks first, then a tiny last
                        # chunk so the closing evacuate+store tail is short
                        parts = [c for _, c in _chunks(cap - 96, reverse=True)]
                        parts.append(96)
                        mchunks, off = [], 0
                        for c in parts:
                            mchunks.append((off, c))
                            off += c
                    for ci, (cs, cw) in enumerate(mchunks):
                        ps = psp.tile([P, cw], f32, tag="ps", name=f"ps{ci}")
                        for kq in range(KT2):
                            nc.tensor.matmul(
                                ps[:],
                                w2m[:, kq, :],
                                hT[:, kq, cs:cs + cw],
                                start=(kq == 0),
                                stop=(kq == KT2 - 1),
                            )
                        # evacuate psum and store; b2 is added on the host
                        # during the combine step. The final phase's stores
                        # alternate between the sync and gpsimd DMA queues so
                        # the kernel tail isn't one serialized store queue
                        # (the gpsimd queue is done with weights by then).
                        ot = op.tile([P, cw], f32, tag="o", name=f"o{ci}")
                        nc.vector.tensor_copy(ot[:], ps[:])
                        if tag == "b":
                            # spread the final phase's stores over three DMA
                            # queues (all idle by then) to shorten the tail
                            q = [nc.sync, nc.gpsimd, nc.scalar][(m + ci) % 3]
                        else:
                            q = nc.sync
                        q.dma_start(dram[f"y{tag}"][m, :, cs:cs + cw], ot[:])

    nc.compile()
    return nc


def _pad16(n):
    return max(256, -(-n // 16) * 16)


def kernel(x, Wg, bg, W1, b1, W2, b2):
    global LAST_RESULT
    _ensure_axon_hooks()
    import ml_dtypes
    from concourse.bass_utils import run_bass_kernel_spmd

    bf = ml_dtypes.bfloat16
    x = np.ascontiguousarray(np.asarray(x, dtype=np.float32))
    Wg = np.asarray(Wg, dtype=np.float32)
    bg = np.asarray(bg, dtype=np.float32)
    W1 = np.asarray(W1, dtype=np.float32)
    b1 = np.asarray(b1, dtype=np.float32)
    W2 = np.asarray(W2, dtype=np.float32)
    b2 = np.asarray(b2, dtype=np.float32)

    B, S, D = x.shape
    T = B * S
    xf = x.reshape(T, D)

    top_idx, top_w = _route(xf, Wg, bg)

    tok_idx, tok_w = [], []
    for e in range(NUM_EXPERTS):
        sel = top_idx == e
        rows = np.nonzero(sel.any(axis=1))[0]
        tok_idx.append(rows)
        tok_w.append((top_w * sel).sum(axis=1)[rows].astype(np.float32))

    loads = [len(r) for r in tok_idx]
    order = np.argsort(-np.asarray(loads), kind="stable")
    # pair hottest with coldest: pair p = (order[p], order[7-p])
    pairs = [(int(order[p]), int(order[NUM_EXPERTS - 1 - p]))
             for p in range(NUM_EXPERTS // 2)]
    capA = _pad16(max(loads[a] for a, _ in pairs))
    capB = _pad16(max(loads[b] for _, b in pairs))

    nc = _build_device_program(capA, capB)

    def prep_x(e, cap, chunks):
        idx_pad = np.zeros(cap, dtype=np.int64)
        idx_pad[:loads[e]] = tok_idx[e]
        xg = xf[idx_pad].T.reshape(KT1, P, cap).transpose(1, 0, 2).astype(bf)
        # chunk-major: one contiguous [P, KT1*cw] block per chunk, with the
        # same chunk boundaries the device program uses
        blocks = [np.ascontiguousarray(xg[:, :, cs:cs + cw]).reshape(P, -1)
                  for cs, cw in chunks]
        return np.ascontiguousarray(np.concatenate(blocks, axis=1))

    def prep_w(e, half):
        fh = slice(half * FH, (half + 1) * FH)
        w1t = np.ascontiguousarray(
            W1[e][:, fh].reshape(KT1, P, MT1, P).transpose(2, 1, 0, 3)
            .reshape(MT1, P, KT1 * P).astype(bf))
        w2t = np.ascontiguousarray(
            W2[e][fh, :].reshape(KT2, P, MT2, P).transpose(2, 1, 0, 3)
            .reshape(MT2, P, KT2 * P).astype(bf))
        b1t = np.ascontiguousarray(b1[e][fh].reshape(MT1, P).T)
        return w1t, w2t, b1t

    in_maps = []
    xg_cache = {}
    for p, (ea, eb) in enumerate(pairs):
        xg_cache[ea] = prep_x(ea, capA, _chunksA(capA))
        xg_cache[eb] = prep_x(eb, capB, _chunks(capB))
        for half in range(2):
            w1a, w2a, b1a = prep_w(ea, half)
            w1b, w2b, b1b = prep_w(eb, half)
            in_maps.append({
                "xga": xg_cache[ea], "w1a": w1a, "w2a": w2a, "b1a": b1a,
                "xgb": xg_cache[eb], "w1b": w1b, "w2b": w2b, "b1b": b1b,
            })

    import os
    trace_cores = None
    if os.environ.get("MOE_TRACE_ALL"):
        trace_cores = list(range(NUM_EXPERTS))
    res = run_bass_kernel_spmd(nc, in_maps, core_ids=list(range(NUM_EXPERTS)),
                               trace_cores=trace_cores)
    LAST_RESULT = res

    out = np.zeros((T, D), dtype=np.float32)
    for p, (ea, eb) in enumerate(pairs):
        for e, key, cap in ((ea, "ya", capA), (eb, "yb", capB)):
            n_e = loads[e]
            if n_e == 0:
                continue
            yT = (res.results[2 * p][key].astype(np.float32)
                  + res.results[2 * p + 1][key].astype(np.float32)).reshape(D, cap)
            out[tok_idx[e]] += tok_w[e][:, None] * (yT[:, :n_e].T + b2[e])
    return out.reshape(B, S, D)
